# revision 23
# baseline (speedup 1.0000x reference)
"""Trainium2 Bass kernel for nn_HardwareOptimizedSpikeProcessor.

Reference semantics (per timestep t):
    acc += (s_t @ (W*mask).T) * 2**scale_exp     # [B, Cout]
    spk  = acc >= 2**threshold_exp
    acc  = acc * (1 - spk)
    out[:, :, t] = spk

Strategy (bit-exact vs the fp32 reference):
  - Shard batch/2 x cout/4: each of the 8 cores handles 32 samples x 512
    output channels.  Matmul is exact: spikes are 0/1 (fp8e4 moving tensor),
    masked weights are ints in [-127,127] (bf16 stationary), PSUM fp32.
  - Mask-block skipping via permutation: the connectivity mask is 8x8-block
    sparse (90%).  A cin-block is dead for a 128-cout m-chunk with prob
    0.9^16 ~ 18.5%.  The host (a) regroups the 256 cout-blocks into 16
    m-chunks clustered so each chunk's common-dead cin-set is large (~77 of
    256), and (b) per cout-shard picks a cin-block permutation that packs 16
    dead blocks per skip-chunk into fixed k-chunk slots.  The program then
    statically skips s_m of the 16 contraction chunks per m (typically
    3+3+3+3 = 12 of 64 -> 18.75% less PE work).  Exact: skipped chunks
    multiply all-zero weights.  Output channels are unscrambled on the host.
  - Weights ship as int8 (halves startup-critical DMA bytes) in half-m
    pieces on the single sync HWDGE queue, interleaved with the first spike
    chunks in demand order; a DVE tensor_scalar converts them to bf16 right
    behind the DMA stream (the DVE is idle until the block-0 scan).
  - Scan step is 2 DVE instructions (~580ns/step):
        u_t  = acc + c_t                         (tensor_tensor add)
        acc  = (u_t < thr) * u_t                 (scalar_tensor_tensor)
    spikes (u_t >= thr) are extracted in 4-step batches on the DVE.  The
    last block's adds read c straight from PSUM, skipping the ACT drain.
  - PE p-state warm-up junk matmuls bridge the framework preamble (~7.5us)
    until the first weights land (~12us), so real matmuls run at full clock.
"""

import sys

for _p in ("/opt/trn_rl_repo",):
    if _p not in sys.path:
        sys.path.insert(0, _p)

import numpy as np
import ml_dtypes

import concourse.bass as bass
import concourse.mybir as mybir
import concourse.tile as tile
from concourse.bass_utils import run_bass_kernel_spmd

B, CIN, COUT, T = 64, 2048, 2048, 128
NCORES = 8
NB = 2                      # batch shards
NQ = 4                      # cout shards
BLOC = B // NB              # 32 samples per core
QLOC = COUT // NQ           # 512 output channels per core
MC = QLOC // 128            # 4 output-channel chunks per core
KC = CIN // 128             # 16 contraction chunks
NGRP = NQ * MC              # 16 cout groups of 16 blocks globally

BLOCKS = [(8 * i, 8) for i in range(16)]
TBMAX = 8
TCHUNKS = [8, 8, 16, 32, 32, 32]
NBLK = len(BLOCKS)
assert sum(TCHUNKS) == T
assert sum(tb for _, tb in BLOCKS) == T

NJUNK = 13                  # p-state warm-up matmuls (~2.8us at mid clock)

_MAX_WAITS = 1


def _split_excess_waits(nc):
    """This container's walrus build accepts at most one sync-wait per
    instruction; spill extra waits onto same-engine NOPs placed before the
    offending instruction."""
    for f in nc.m.functions:
        for bb in f.blocks:
            new_list = []
            for ins in bb.instructions:
                si = ins.sync_info
                waits = list(si.on_wait) if si is not None and si.on_wait else []
                if len(waits) > _MAX_WAITS:
                    extra, keep = waits[:-_MAX_WAITS], waits[-_MAX_WAITS:]
                    for i in range(0, len(extra), _MAX_WAITS):
                        nop = mybir.InstNoOp(
                            name=f"{ins.name}-waitsplit-{i}", ins=[], outs=[]
                        )
                        nop.engine = ins.engine
                        nop.sync_info = mybir.SyncInfo(
                            on_wait=extra[i : i + _MAX_WAITS], on_update=[]
                        )
                        new_list.append(nop)
                    ins.sync_info = mybir.SyncInfo(
                        on_wait=keep,
                        on_update=list(si.on_update) if si.on_update else [],
                    )
                new_list.append(ins)
            bb.instructions[:] = new_list


def _skip_slots(pattern):
    """Fixed, disjoint skip-chunk slots per m: m=0 takes the highest s_0
    chunk indices, m=1 the next ones down, etc."""
    slots, hi = [], KC
    for s in pattern:
        slots.append(list(range(hi - s, hi)))
        hi -= s
    return slots


def _build(thr: float, pattern: tuple):
    f32 = mybir.dt.float32
    bf16 = mybir.dt.bfloat16
    fp8 = mybir.dt.float8e4
    u8 = mybir.dt.uint8
    i8 = mybir.dt.int8
    nc = bass.Bass()

    skip = _skip_slots(pattern)
    live_k = [[k for k in range(KC) if k not in skip[m]] for m in range(MC)]
    nk = [len(lk) for lk in live_k]
    off = [sum(nk[:m]) for m in range(MC)]
    nktot = sum(nk)

    # W^T per m-chunk [cin_lo, nk_m, cout_lo], int8 (2**scale_exp folded in)
    wt_ds = [
        nc.dram_tensor(f"wt{m}", [128, nk[m], 128], i8, kind="ExternalInput")
        for m in range(MC)
    ]
    # spike chunks, each contiguous [cin_lo, k, b, tc]
    spk_ds = [
        nc.dram_tensor(f"spk{j}", [128, KC, BLOC, tc], fp8, kind="ExternalInput")
        for j, tc in enumerate(TCHUNKS)
    ]
    # per-block spike outputs [cout_lo, t, m, b]
    out_ds = [
        nc.dram_tensor(f"out{j}", [128, tb, MC, BLOC], u8, kind="ExternalOutput")
        for j, (_, tb) in enumerate(BLOCKS)
    ]

    # block -> (chunk index, t offset within chunk)
    cstart = []
    s = 0
    for tc in TCHUNKS:
        cstart.append(s)
        s += tc
    blk_map = []
    for t0, tb in BLOCKS:
        cj = max(i for i, cs in enumerate(cstart) if cs <= t0)
        assert t0 + tb <= cstart[cj] + TCHUNKS[cj]
        blk_map.append((cj, t0 - cstart[cj]))

    with tile.TileContext(nc) as tc:
        with (
            tc.tile_pool(name="const", bufs=1) as const,
            tc.tile_pool(name="cpool", bufs=3) as cpool,
            tc.tile_pool(name="upool", bufs=3) as upool,
            tc.tile_pool(name="opool", bufs=3) as opool,
            tc.tile_pool(name="psum", bufs=3, space="PSUM") as psum,
        ):
            wt_sb = const.tile([128, nktot, 128], bf16)
            wt_i8 = const.tile([128, nktot, 128], i8)
            spk_sbs = [
                const.tile([128, KC, BLOC, tc], fp8, name=f"spk_sb{j}")
                for j, tc in enumerate(TCHUNKS)
            ]
            acc = const.tile([128, MC, BLOC], f32)
            junk = const.tile([128, 256], bf16)

            # All input DMAs ride the single sync HWDGE queue in demand order
            # (a second HWDGE queue or the gpsimd swdge queue both measured
            # ~100GB/s and stole bandwidth from q1).  Weights ship as int8 in
            # half-m pieces that a DVE convert pipelines to bf16.
            nc.gpsimd.memset(junk[:], 0.0)
            nc.vector.memset(acc[:], 0.0)
            halves = []
            for m in range(MC):
                h0 = nk[m] // 2
                halves.append([(off[m], h0, 0), (off[m] + h0, nk[m] - h0, h0)])
            nc.sync.dma_start(spk_sbs[0][:, :8], spk_ds[0][:, :8])
            for m in range(MC):
                for o, n, src in halves[m]:
                    nc.sync.dma_start(
                        wt_i8[:, o : o + n], wt_ds[m][:, src : src + n]
                    )
                if m == 0:
                    nc.sync.dma_start(spk_sbs[0][:, 8:], spk_ds[0][:, 8:])
            nc.sync.dma_start(spk_sbs[1][:], spk_ds[1][:])
            # int8->bf16 converts on the ACT engine (idle until the block-0
            # PSUM drain; keeps the DVE free for the scan chain)
            for m in range(MC):
                for o, n, _ in halves[m]:
                    nc.scalar.copy(wt_sb[:, o : o + n], wt_i8[:, o : o + n])

            # block index at which to issue each remaining chunk's DMA
            chunk_issue = {}
            for cidx in range(2, len(TCHUNKS)):
                first_blk = next(
                    jj for jj, (cj2, _) in enumerate(blk_map) if cj2 == cidx
                )
                chunk_issue.setdefault(max(0, first_blk - 2), []).append(cidx)

            # PE p-state warm-up on junk data while the DMAs land
            wps = psum.tile([128, MC, BLOC * TBMAX], f32, tag="ps", name="ps")
            for _ in range(NJUNK):
                nc.tensor.matmul(wps[:, 0, :256], lhsT=junk[:, :128], rhs=junk[:])

            for j, (t0, tb) in enumerate(BLOCKS):
                for cidx in chunk_issue.get(j, []):
                    nc.sync.dma_start(spk_sbs[cidx][:], spk_ds[cidx][:])
                cj, toff = blk_map[j]
                nfree = BLOC * tb
                ps = psum.tile([128, MC, BLOC * TBMAX], f32, tag="ps", name="ps")
                for m in range(MC):
                    for ki, k in enumerate(live_k[m]):
                        nc.tensor.matmul(
                            ps[:, m, :nfree],
                            lhsT=wt_sb[:, off[m] + ki, :],
                            rhs=spk_sbs[cj][:, k, :, toff : toff + tb],
                            start=(ki == 0),
                            stop=(ki == nk[m] - 1),
                        )
                # PSUM [p, m, (b t)] -> SBUF c [p, t, m, b] so each scan step
                # reads a contiguous [128, (m b)] slice.
                last = j == NBLK - 1
                ps_v = ps[:, :, :nfree].rearrange("p m (b t) -> p m b t", b=BLOC)
                H = tb // 2
                ngr = tb // H
                if not last:
                    c = cpool.tile([128, TBMAX, MC, BLOC], f32, tag="cblk")
                    for h in range(ngr):
                        nc.scalar.copy(
                            c[:, h * H : (h + 1) * H].rearrange(
                                "p t m b -> p m b t"
                            ),
                            ps_v[:, :, :, h * H : (h + 1) * H],
                        )
                u = upool.tile([128, TBMAX, MC, BLOC], f32, tag="ublk")
                ob = opool.tile([128, TBMAX, MC, BLOC], u8, tag="oblk")
                for t in range(tb):
                    c_t = ps_v[:, :, :, t] if last else c[:, t]
                    nc.vector.tensor_tensor(
                        u[:, t], acc[:], c_t, mybir.AluOpType.add
                    )
                    # the reset after the very last timestep is dead code
                    if not (last and t == tb - 1):
                        nc.vector.scalar_tensor_tensor(
                            acc[:], u[:, t], thr, u[:, t],
                            mybir.AluOpType.is_lt, mybir.AluOpType.mult,
                        )
                # spikes = (u >= thr), once per block off the serial chain
                nc.vector.tensor_scalar(
                    ob[:, :tb], u[:, :tb], thr, None, mybir.AluOpType.is_ge
                )
                nc.sync.dma_start(out_ds[j][:], ob[:, :tb])

    _split_excess_waits(nc)
    return nc


# ---------------------------------------------------------------------------
# Host-side mask analysis: regroup cout-blocks + pack dead cin-blocks into
# fixed skip chunks (see module docstring).

PATTERNS = [
    (4, 4, 4, 2), (4, 4, 3, 3), (4, 4, 4, 1), (4, 4, 3, 2), (4, 3, 3, 3),
    (4, 4, 2, 2), (4, 3, 3, 2), (3, 3, 3, 3), (4, 3, 2, 2), (3, 3, 3, 2),
    (3, 3, 2, 2), (3, 3, 3, 1), (3, 2, 2, 2), (3, 3, 2, 1), (2, 2, 2, 2),
    (3, 2, 2, 1), (2, 2, 2, 1), (2, 2, 1, 1), (2, 1, 1, 1), (1, 1, 1, 1),
    (0, 0, 0, 0),
]


def _build_groups(zero, seed_order):
    """Round-robin greedy: grow 16 groups of 16 cout-blocks, maximizing each
    group's common-dead cin-block set."""
    remaining = set(range(NGRP * 16))
    seeds = [c for c in seed_order if c in remaining][: NGRP]
    groups = [[s] for s in seeds]
    inters = [zero[s].copy() for s in seeds]
    for s in seeds:
        remaining.discard(s)
    for _ in range(15):
        for g in range(NGRP):
            rem = sorted(remaining)
            best = max(rem, key=lambda c: int((inters[g] & zero[c]).sum()))
            groups[g].append(best)
            inters[g] &= zero[best]
            remaining.discard(best)
    return groups, inters


def _pack_shards(inters, shard_assign, pattern):
    """Per shard, pick disjoint sets of 16*s_m cin-blocks dead for group m.
    Exact feasibility via Kuhn's bipartite matching (group-slots x blocks).
    Returns per-shard {group: (s, blocks)} or None."""
    out = []
    for sh in shard_assign:
        gs = sorted(sh, key=lambda gi: -int(inters[gi].sum()))
        slots = []  # slot -> group index position
        for pos, s in enumerate(pattern):
            slots += [pos] * (16 * s)
        adj = [np.where(inters[gs[pos]])[0].tolist() for pos in range(4)]
        match_b = {}  # block -> slot

        def try_slot(si, visited):
            for b in adj[slots[si]]:
                if b in visited:
                    continue
                visited.add(b)
                if b not in match_b or try_slot(match_b[b], visited):
                    match_b[b] = si
                    return True
            return False

        sys.setrecursionlimit(10000)
        ok = True
        # process scarcest groups' slots first
        order = sorted(range(len(slots)), key=lambda si: len(adj[slots[si]]))
        for si in order:
            if not try_slot(si, set()):
                ok = False
                break
        if not ok:
            return None
        asg = {}
        for pos, gi in enumerate(gs):
            blks = np.array(sorted(b for b, si in match_b.items() if slots[si] == pos))
            assert len(blks) == 16 * pattern[pos]
            asg[gi] = (pattern[pos], blks)
        out.append(asg)
    return out


def _plan(wm):
    """From masked weights, choose (pattern, per-core cout order, per-shard
    cin permutation, per-shard m-group order)."""
    zero = (
        wm.reshape(COUT // 8, 8, CIN // 8, 8) == 0
    ).all(axis=(1, 3))  # [cout_blk, cin_blk] all-zero 8x8 blocks
    rng = np.random.default_rng(0)
    plans = []
    for trial in range(6):
        order = list(rng.permutation(NGRP * 16))
        groups, inters = _build_groups(zero, order)
        sizes = [int(i.sum()) for i in inters]
        order_g = np.argsort(sizes)[::-1]
        for variant in range(2):
            shard_assign = [[] for _ in range(NQ)]
            for i, gi in enumerate(order_g):
                if variant == 0:
                    s_idx = int(i % NQ)
                else:
                    s_idx = int(i % NQ) if (i // NQ) % 2 == 0 else NQ - 1 - int(i % NQ)
                shard_assign[s_idx].append(int(gi))
            plans.append((groups, inters, shard_assign))
    for pattern in PATTERNS:
        for groups, inters, shard_assign in plans:
            packs = _pack_shards(inters, shard_assign, pattern)
            if packs is None:
                continue
            skip = _skip_slots(pattern)
            shard_plans = []
            ok = True
            for cq in range(NQ):
                asg = packs[cq]
                # m order: groups by quota desc (pattern order)
                gs = sorted(asg.keys(), key=lambda gi: (-asg[gi][0],))
                perm = np.full(KC * 16, -1, np.int64)  # slot -> cin_blk
                used = np.zeros(NGRP * 16, bool)
                for m, gi in enumerate(gs):
                    s, blks = asg[gi]
                    if s != pattern[m]:
                        ok = False
                    slots = [sl for k in skip[m] for sl in range(k * 16, k * 16 + 16)]
                    perm[slots] = blks
                    used[blks] = True
                rest = np.where(~used)[0]
                perm[perm == -1] = rest
                coutblks = [b for gi in gs for b in groups[gi]]
                shard_plans.append((perm, coutblks))
            if ok:
                return pattern, shard_plans
    return (0, 0, 0, 0), [
        (np.arange(KC * 16), list(range(cq * 64, (cq + 1) * 64)))
        for cq in range(NQ)
    ]


def _prep_inputs(spikes, weights, mask, scale_exp):
    wm = weights * mask  # integers <= 127, exact
    scale = np.exp2(scale_exp.astype(np.float64)).astype(np.float32)
    wm = wm * scale[:, None]  # fold power-of-2 scale in
    pattern, shard_plans = _plan(wm)
    skip = _skip_slots(pattern)
    live_k = [[k for k in range(KC) if k not in skip[m]] for m in range(MC)]

    in_maps = []
    gather = []  # per core: cout channel indices in program order
    for core in range(NCORES):
        bh, cq = divmod(core, NQ)
        perm, coutblks = shard_plans[cq]
        cin_idx = (np.asarray(perm)[:, None] * 8 + np.arange(8)[None, :]).ravel()
        cout_idx = (np.asarray(coutblks)[:, None] * 8 + np.arange(8)[None, :]).ravel()
        gather.append(cout_idx)
        # weights: [qloc, cin] -> permuted/grouped -> per m [cin_lo, nk, cout_lo]
        wq = wm[cout_idx][:, cin_idx]  # [512, 2048]
        m_map = {}
        wqk = wq.T.reshape(KC, 128, QLOC)  # [k, cin_lo, cout]
        for m in range(MC):
            wqm = wqk[:, :, m * 128 : (m + 1) * 128]  # [k, cin_lo, cout_lo]
            for k in skip[m]:
                assert not wqm[k].any(), "skip chunk not all-zero; bad packing"
            wtm = wqm[live_k[m]].transpose(1, 0, 2)  # [cin_lo, nk, cout_lo]
            m_map[f"wt{m}"] = np.ascontiguousarray(wtm.astype(np.int8))
        # spikes for this batch shard: [b, cin, t] -> permuted [cin_lo, k, b, t]
        s = spikes[bh * BLOC : (bh + 1) * BLOC]
        sp = s.transpose(1, 0, 2)[cin_idx]  # [cin, b, t]
        a = sp.reshape(KC, 128, BLOC, T).transpose(1, 0, 2, 3)
        a = a.astype(ml_dtypes.float8_e4m3)
        t0 = 0
        for jj, tc in enumerate(TCHUNKS):
            m_map[f"spk{jj}"] = np.ascontiguousarray(a[:, :, :, t0 : t0 + tc])
            t0 += tc
        in_maps.append(m_map)
    return pattern, in_maps, gather


_CACHE = {}


def _get_program(thr: float, pattern: tuple):
    key = (thr, pattern)
    if key not in _CACHE:
        _CACHE[key] = _build(thr, pattern)
    return _CACHE[key]


def kernel(spikes, weights, mask, scale_exp, threshold_exp, **run_kwargs):
    thr = float(2.0 ** int(np.asarray(threshold_exp)))
    pattern, in_maps, gather = _prep_inputs(
        np.asarray(spikes, dtype=np.float32),
        np.asarray(weights, dtype=np.float32),
        np.asarray(mask, dtype=np.float32),
        np.asarray(scale_exp),
    )
    nc = _get_program(thr, pattern)
    res = run_bass_kernel_spmd(
        nc, in_maps, core_ids=list(range(NCORES)), **run_kwargs
    )
    full = np.zeros((B, COUT, T), dtype=np.float32)
    for core in range(NCORES):
        bh, cq = divmod(core, NQ)
        blks = [
            np.asarray(res.results[core][f"out{j}"]) for j in range(NBLK)
        ]  # each [cout_lo, t, m, b]
        a = np.concatenate(blks, axis=1)  # [cout_lo, T, m, b]
        # -> [b, m, cout_lo, T] -> [b_loc, qloc, T]
        a = a.transpose(3, 2, 0, 1).reshape(BLOC, QLOC, T)
        full[bh * BLOC : (bh + 1) * BLOC, gather[core]] = a
    if run_kwargs:
        return full, res
    return full


# revision 27
# speedup vs baseline: 1.0289x; 1.0289x over previous
"""Trainium2 Bass kernel for nn_HardwareOptimizedSpikeProcessor.

Reference semantics (per timestep t):
    acc += (s_t @ (W*mask).T) * 2**scale_exp     # [B, Cout]
    spk  = acc >= 2**threshold_exp
    acc  = acc * (1 - spk)
    out[:, :, t] = spk

Strategy (bit-exact vs the fp32 reference):
  - Shard batch/2 x cout/4: each of the 8 cores handles 32 samples x 512
    output channels.  Matmul is exact: spikes are 0/1 (fp8e4 moving tensor),
    masked weights are ints in [-127,127] (bf16 stationary), PSUM fp32.
  - Mask-block skipping via permutation: the connectivity mask is 8x8-block
    sparse (90%).  A cin-block is dead for a 128-cout m-chunk with prob
    0.9^16 ~ 18.5%.  The host (a) regroups the 256 cout-blocks into 16
    m-chunks clustered so each chunk's common-dead cin-set is large (~77 of
    256), and (b) per cout-shard picks a cin-block permutation that packs 16
    dead blocks per skip-chunk into fixed k-chunk slots.  The program then
    statically skips s_m of the 16 contraction chunks per m (typically
    3+3+3+3 = 12 of 64 -> 18.75% less PE work).  Exact: skipped chunks
    multiply all-zero weights.  Output channels are unscrambled on the host.
  - Weights ship as int8 (halves startup-critical DMA bytes) in half-m
    pieces on the single sync HWDGE queue, interleaved with the first spike
    chunks in demand order; a DVE tensor_scalar converts them to bf16 right
    behind the DMA stream (the DVE is idle until the block-0 scan).
  - Scan step is 2 DVE instructions (~580ns/step):
        u_t  = acc + c_t                         (tensor_tensor add)
        acc  = (u_t < thr) * u_t                 (scalar_tensor_tensor)
    spikes (u_t >= thr) are extracted in 4-step batches on the DVE.  The
    last block's adds read c straight from PSUM, skipping the ACT drain.
  - PE p-state warm-up junk matmuls bridge the framework preamble (~7.5us)
    until the first weights land (~12us), so real matmuls run at full clock.
"""

import sys

for _p in ("/opt/trn_rl_repo",):
    if _p not in sys.path:
        sys.path.insert(0, _p)

import numpy as np
import ml_dtypes

import concourse.bass as bass
import concourse.mybir as mybir
import concourse.tile as tile
from concourse.bass_utils import run_bass_kernel_spmd

B, CIN, COUT, T = 64, 2048, 2048, 128
NCORES = 8
NB = 2                      # batch shards
NQ = 4                      # cout shards
BLOC = B // NB              # 32 samples per core
QLOC = COUT // NQ           # 512 output channels per core
MC = QLOC // 128            # 4 output-channel chunks per core
KC = CIN // 128             # 16 contraction chunks
NGRP = NQ * MC              # 16 cout groups of 16 blocks globally

BLOCKS = [(8 * i, 8) for i in range(16)]
TBMAX = 8
TCHUNKS = [8, 8, 16, 32, 32, 32]
NBLK = len(BLOCKS)
assert sum(TCHUNKS) == T
assert sum(tb for _, tb in BLOCKS) == T

NJUNK = 15                  # p-state warm-up matmuls (~3.2us at mid clock)

_MAX_WAITS = 1


def _split_excess_waits(nc):
    """This container's walrus build accepts at most one sync-wait per
    instruction; spill extra waits onto same-engine NOPs placed before the
    offending instruction."""
    for f in nc.m.functions:
        for bb in f.blocks:
            new_list = []
            for ins in bb.instructions:
                si = ins.sync_info
                waits = list(si.on_wait) if si is not None and si.on_wait else []
                if len(waits) > _MAX_WAITS:
                    extra, keep = waits[:-_MAX_WAITS], waits[-_MAX_WAITS:]
                    for i in range(0, len(extra), _MAX_WAITS):
                        nop = mybir.InstNoOp(
                            name=f"{ins.name}-waitsplit-{i}", ins=[], outs=[]
                        )
                        nop.engine = ins.engine
                        nop.sync_info = mybir.SyncInfo(
                            on_wait=extra[i : i + _MAX_WAITS], on_update=[]
                        )
                        new_list.append(nop)
                    ins.sync_info = mybir.SyncInfo(
                        on_wait=keep,
                        on_update=list(si.on_update) if si.on_update else [],
                    )
                new_list.append(ins)
            bb.instructions[:] = new_list


def _skip_slots(pattern):
    """Fixed, disjoint skip-chunk slots per m: m=0 takes the highest s_0
    chunk indices, m=1 the next ones down, etc."""
    slots, hi = [], KC
    for s in pattern:
        slots.append(list(range(hi - s, hi)))
        hi -= s
    return slots


def _build(thr: float, pattern: tuple):
    f32 = mybir.dt.float32
    bf16 = mybir.dt.bfloat16
    fp8 = mybir.dt.float8e4
    u8 = mybir.dt.uint8
    i8 = mybir.dt.int8
    nc = bass.Bass()

    skip = _skip_slots(pattern)
    live_k = [[k for k in range(KC) if k not in skip[m]] for m in range(MC)]
    nk = [len(lk) for lk in live_k]
    off = [sum(nk[:m]) for m in range(MC)]
    nktot = sum(nk)

    # W^T per m-chunk [cin_lo, nk_m, cout_lo], int8 (2**scale_exp folded in)
    wt_ds = [
        nc.dram_tensor(f"wt{m}", [128, nk[m], 128], i8, kind="ExternalInput")
        for m in range(MC)
    ]
    # spike chunks, each contiguous [cin_lo, k, b, tc]
    spk_ds = [
        nc.dram_tensor(f"spk{j}", [128, KC, BLOC, tc], fp8, kind="ExternalInput")
        for j, tc in enumerate(TCHUNKS)
    ]
    # per-block spike outputs [cout_lo, t, m, b]
    out_ds = [
        nc.dram_tensor(f"out{j}", [128, tb, MC, BLOC], u8, kind="ExternalOutput")
        for j, (_, tb) in enumerate(BLOCKS)
    ]

    # block -> (chunk index, t offset within chunk)
    cstart = []
    s = 0
    for tc in TCHUNKS:
        cstart.append(s)
        s += tc
    blk_map = []
    for t0, tb in BLOCKS:
        cj = max(i for i, cs in enumerate(cstart) if cs <= t0)
        assert t0 + tb <= cstart[cj] + TCHUNKS[cj]
        blk_map.append((cj, t0 - cstart[cj]))

    with tile.TileContext(nc) as tc:
        with (
            tc.tile_pool(name="const", bufs=1) as const,
            tc.tile_pool(name="cpool", bufs=3) as cpool,
            tc.tile_pool(name="upool", bufs=3) as upool,
            tc.tile_pool(name="opool", bufs=3) as opool,
            tc.tile_pool(name="upool2", bufs=2) as upool2,
            tc.tile_pool(name="opool2", bufs=2) as opool2,
            tc.tile_pool(name="psum", bufs=3, space="PSUM") as psum,
        ):
            wt_sb = const.tile([128, nktot, 128], bf16)
            wt_i8 = const.tile([128, nktot, 128], i8)
            spk_sbs = [
                const.tile([128, KC, BLOC, tc], fp8, name=f"spk_sb{j}")
                for j, tc in enumerate(TCHUNKS)
            ]
            acc = const.tile([128, MC, BLOC], f32)
            junk = const.tile([128, 256], bf16)

            # All input DMAs ride the single sync HWDGE queue in demand order
            # (a second HWDGE queue or the gpsimd swdge queue both measured
            # ~100GB/s and stole bandwidth from q1).  Weights ship as int8 in
            # half-m pieces that a DVE convert pipelines to bf16.
            nc.gpsimd.memset(junk[:], 0.0)
            nc.vector.memset(acc[:], 0.0)
            halves = []
            for m in range(MC):
                h0 = nk[m] // 2
                halves.append([(off[m], h0, 0), (off[m] + h0, nk[m] - h0, h0)])
            nc.sync.dma_start(spk_sbs[0][:, :8], spk_ds[0][:, :8])
            for m in range(MC):
                for o, n, src in halves[m]:
                    nc.sync.dma_start(
                        wt_i8[:, o : o + n], wt_ds[m][:, src : src + n]
                    )
                if m == 0:
                    nc.sync.dma_start(spk_sbs[0][:, 8:], spk_ds[0][:, 8:])
            nc.sync.dma_start(spk_sbs[1][:], spk_ds[1][:])
            # int8->bf16 converts on the DVE (idle until the block-0 scan;
            # the ACT engine is ~1.1us/op fixed-cost and would add latency)
            for m in range(MC):
                for o, n, _ in halves[m]:
                    nc.vector.tensor_scalar(
                        wt_sb[:, o : o + n], wt_i8[:, o : o + n], 0.0, None,
                        mybir.AluOpType.add,
                    )

            # block index at which to issue each remaining chunk's DMA
            chunk_issue = {}
            for cidx in range(2, len(TCHUNKS)):
                first_blk = next(
                    jj for jj, (cj2, _) in enumerate(blk_map) if cj2 == cidx
                )
                chunk_issue.setdefault(max(0, first_blk - 2), []).append(cidx)

            # PE p-state warm-up on junk data while the DMAs land
            wps = psum.tile([128, MC, BLOC * TBMAX], f32, tag="ps", name="ps")
            for _ in range(NJUNK):
                nc.tensor.matmul(wps[:, 0, :256], lhsT=junk[:, :128], rhs=junk[:])

            for j, (t0, tb) in enumerate(BLOCKS):
                for cidx in chunk_issue.get(j, []):
                    nc.sync.dma_start(spk_sbs[cidx][:], spk_ds[cidx][:])
                cj, toff = blk_map[j]
                nfree = BLOC * tb
                ps = psum.tile([128, MC, BLOC * TBMAX], f32, tag="ps", name="ps")
                for m in range(MC):
                    for ki, k in enumerate(live_k[m]):
                        nc.tensor.matmul(
                            ps[:, m, :nfree],
                            lhsT=wt_sb[:, off[m] + ki, :],
                            rhs=spk_sbs[cj][:, k, :, toff : toff + tb],
                            start=(ki == 0),
                            stop=(ki == nk[m] - 1),
                        )
                # PSUM [p, m, (b t)] -> SBUF c [p, t, m, b] so each scan step
                # reads a contiguous [128, (m b)] slice.
                last = j == NBLK - 1
                ps_v = ps[:, :, :nfree].rearrange("p m (b t) -> p m b t", b=BLOC)
                H = tb // 2
                ngr = tb // H
                if not last:
                    c = cpool.tile([128, TBMAX, MC, BLOC], f32, tag="cblk")
                    for h in range(ngr):
                        nc.scalar.copy(
                            c[:, h * H : (h + 1) * H].rearrange(
                                "p t m b -> p m b t"
                            ),
                            ps_v[:, :, :, h * H : (h + 1) * H],
                        )
                if j < NBLK - 2:
                    # paired extract: one is_ge per two blocks keeps the DVE
                    # chain cadence below the PE block cadence
                    if j % 2 == 0:
                        u_pair = upool.tile(
                            [128, 2 * TBMAX, MC, BLOC], f32, tag="upair"
                        )
                    uo = (j % 2) * TBMAX
                    for t in range(tb):
                        nc.vector.tensor_tensor(
                            u_pair[:, uo + t], acc[:], c[:, t],
                            mybir.AluOpType.add,
                        )
                        nc.vector.scalar_tensor_tensor(
                            acc[:], u_pair[:, uo + t], thr, u_pair[:, uo + t],
                            mybir.AluOpType.is_lt, mybir.AluOpType.mult,
                        )
                    if j % 2 == 1:
                        ob_pair = opool.tile(
                            [128, 2 * TBMAX, MC, BLOC], u8, tag="opair"
                        )
                        nc.vector.tensor_scalar(
                            ob_pair[:], u_pair[:], thr, None,
                            mybir.AluOpType.is_ge,
                        )
                        nc.sync.dma_start(out_ds[j - 1][:], ob_pair[:, :TBMAX])
                        nc.sync.dma_start(out_ds[j][:], ob_pair[:, TBMAX:])
                else:
                    # last two blocks: fine-grained extracts to shorten the
                    # post-matmul tail
                    u = upool2.tile([128, TBMAX, MC, BLOC], f32, tag="ublk")
                    ob = opool2.tile([128, TBMAX, MC, BLOC], u8, tag="oblk")
                    for h in range(ngr):
                        for t in range(h * H, (h + 1) * H):
                            c_t = ps_v[:, :, :, t] if last else c[:, t]
                            nc.vector.tensor_tensor(
                                u[:, t], acc[:], c_t, mybir.AluOpType.add
                            )
                            # reset after the very last timestep is dead code
                            if not (last and t == tb - 1):
                                nc.vector.scalar_tensor_tensor(
                                    acc[:], u[:, t], thr, u[:, t],
                                    mybir.AluOpType.is_lt, mybir.AluOpType.mult,
                                )
                        nc.vector.tensor_scalar(
                            ob[:, h * H : (h + 1) * H],
                            u[:, h * H : (h + 1) * H],
                            thr, None, mybir.AluOpType.is_ge,
                        )
                        nc.sync.dma_start(
                            out_ds[j][:, h * H : (h + 1) * H],
                            ob[:, h * H : (h + 1) * H],
                        )

    _split_excess_waits(nc)
    return nc


# ---------------------------------------------------------------------------
# Host-side mask analysis: regroup cout-blocks + pack dead cin-blocks into
# fixed skip chunks (see module docstring).

PATTERNS = [
    (4, 4, 4, 2), (4, 4, 3, 3), (4, 4, 4, 1), (4, 4, 3, 2), (4, 3, 3, 3),
    (4, 4, 2, 2), (4, 3, 3, 2), (3, 3, 3, 3), (4, 3, 2, 2), (3, 3, 3, 2),
    (3, 3, 2, 2), (3, 3, 3, 1), (3, 2, 2, 2), (3, 3, 2, 1), (2, 2, 2, 2),
    (3, 2, 2, 1), (2, 2, 2, 1), (2, 2, 1, 1), (2, 1, 1, 1), (1, 1, 1, 1),
    (0, 0, 0, 0),
]


def _build_groups(zero, seed_order):
    """Round-robin greedy: grow 16 groups of 16 cout-blocks, maximizing each
    group's common-dead cin-block set."""
    remaining = set(range(NGRP * 16))
    seeds = [c for c in seed_order if c in remaining][: NGRP]
    groups = [[s] for s in seeds]
    inters = [zero[s].copy() for s in seeds]
    for s in seeds:
        remaining.discard(s)
    for _ in range(15):
        for g in range(NGRP):
            rem = sorted(remaining)
            best = max(rem, key=lambda c: int((inters[g] & zero[c]).sum()))
            groups[g].append(best)
            inters[g] &= zero[best]
            remaining.discard(best)
    return groups, inters


def _pack_shards(inters, shard_assign, pattern):
    """Per shard, pick disjoint sets of 16*s_m cin-blocks dead for group m.
    Exact feasibility via Kuhn's bipartite matching (group-slots x blocks).
    Returns per-shard {group: (s, blocks)} or None."""
    out = []
    for sh in shard_assign:
        gs = sorted(sh, key=lambda gi: -int(inters[gi].sum()))
        slots = []  # slot -> group index position
        for pos, s in enumerate(pattern):
            slots += [pos] * (16 * s)
        adj = [np.where(inters[gs[pos]])[0].tolist() for pos in range(4)]
        match_b = {}  # block -> slot

        def try_slot(si, visited):
            for b in adj[slots[si]]:
                if b in visited:
                    continue
                visited.add(b)
                if b not in match_b or try_slot(match_b[b], visited):
                    match_b[b] = si
                    return True
            return False

        sys.setrecursionlimit(10000)
        ok = True
        # process scarcest groups' slots first
        order = sorted(range(len(slots)), key=lambda si: len(adj[slots[si]]))
        for si in order:
            if not try_slot(si, set()):
                ok = False
                break
        if not ok:
            return None
        asg = {}
        for pos, gi in enumerate(gs):
            blks = np.array(sorted(b for b, si in match_b.items() if slots[si] == pos))
            assert len(blks) == 16 * pattern[pos]
            asg[gi] = (pattern[pos], blks)
        out.append(asg)
    return out


def _plan(wm):
    """From masked weights, choose (pattern, per-core cout order, per-shard
    cin permutation, per-shard m-group order)."""
    zero = (
        wm.reshape(COUT // 8, 8, CIN // 8, 8) == 0
    ).all(axis=(1, 3))  # [cout_blk, cin_blk] all-zero 8x8 blocks
    rng = np.random.default_rng(0)
    plans = []
    for trial in range(6):
        order = list(rng.permutation(NGRP * 16))
        groups, inters = _build_groups(zero, order)
        sizes = [int(i.sum()) for i in inters]
        order_g = np.argsort(sizes)[::-1]
        for variant in range(2):
            shard_assign = [[] for _ in range(NQ)]
            for i, gi in enumerate(order_g):
                if variant == 0:
                    s_idx = int(i % NQ)
                else:
                    s_idx = int(i % NQ) if (i // NQ) % 2 == 0 else NQ - 1 - int(i % NQ)
                shard_assign[s_idx].append(int(gi))
            plans.append((groups, inters, shard_assign))
    for pattern in PATTERNS:
        for groups, inters, shard_assign in plans:
            packs = _pack_shards(inters, shard_assign, pattern)
            if packs is None:
                continue
            skip = _skip_slots(pattern)
            shard_plans = []
            ok = True
            for cq in range(NQ):
                asg = packs[cq]
                # m order: groups by quota desc (pattern order)
                gs = sorted(asg.keys(), key=lambda gi: (-asg[gi][0],))
                perm = np.full(KC * 16, -1, np.int64)  # slot -> cin_blk
                used = np.zeros(NGRP * 16, bool)
                for m, gi in enumerate(gs):
                    s, blks = asg[gi]
                    if s != pattern[m]:
                        ok = False
                    slots = [sl for k in skip[m] for sl in range(k * 16, k * 16 + 16)]
                    perm[slots] = blks
                    used[blks] = True
                rest = np.where(~used)[0]
                perm[perm == -1] = rest
                coutblks = [b for gi in gs for b in groups[gi]]
                shard_plans.append((perm, coutblks))
            if ok:
                return pattern, shard_plans
    return (0, 0, 0, 0), [
        (np.arange(KC * 16), list(range(cq * 64, (cq + 1) * 64)))
        for cq in range(NQ)
    ]


def _prep_inputs(spikes, weights, mask, scale_exp):
    wm = weights * mask  # integers <= 127, exact
    scale = np.exp2(scale_exp.astype(np.float64)).astype(np.float32)
    wm = wm * scale[:, None]  # fold power-of-2 scale in
    pattern, shard_plans = _plan(wm)
    skip = _skip_slots(pattern)
    live_k = [[k for k in range(KC) if k not in skip[m]] for m in range(MC)]

    in_maps = []
    gather = []  # per core: cout channel indices in program order
    for core in range(NCORES):
        bh, cq = divmod(core, NQ)
        perm, coutblks = shard_plans[cq]
        cin_idx = (np.asarray(perm)[:, None] * 8 + np.arange(8)[None, :]).ravel()
        cout_idx = (np.asarray(coutblks)[:, None] * 8 + np.arange(8)[None, :]).ravel()
        gather.append(cout_idx)
        # weights: [qloc, cin] -> permuted/grouped -> per m [cin_lo, nk, cout_lo]
        wq = wm[cout_idx][:, cin_idx]  # [512, 2048]
        m_map = {}
        wqk = wq.T.reshape(KC, 128, QLOC)  # [k, cin_lo, cout]
        for m in range(MC):
            wqm = wqk[:, :, m * 128 : (m + 1) * 128]  # [k, cin_lo, cout_lo]
            for k in skip[m]:
                assert not wqm[k].any(), "skip chunk not all-zero; bad packing"
            wtm = wqm[live_k[m]].transpose(1, 0, 2)  # [cin_lo, nk, cout_lo]
            m_map[f"wt{m}"] = np.ascontiguousarray(wtm.astype(np.int8))
        # spikes for this batch shard: [b, cin, t] -> permuted [cin_lo, k, b, t]
        s = spikes[bh * BLOC : (bh + 1) * BLOC]
        sp = s.transpose(1, 0, 2)[cin_idx]  # [cin, b, t]
        a = sp.reshape(KC, 128, BLOC, T).transpose(1, 0, 2, 3)
        a = a.astype(ml_dtypes.float8_e4m3)
        t0 = 0
        for jj, tc in enumerate(TCHUNKS):
            m_map[f"spk{jj}"] = np.ascontiguousarray(a[:, :, :, t0 : t0 + tc])
            t0 += tc
        in_maps.append(m_map)
    return pattern, in_maps, gather


_CACHE = {}


def _get_program(thr: float, pattern: tuple):
    key = (thr, pattern)
    if key not in _CACHE:
        _CACHE[key] = _build(thr, pattern)
    return _CACHE[key]


def kernel(spikes, weights, mask, scale_exp, threshold_exp, **run_kwargs):
    thr = float(2.0 ** int(np.asarray(threshold_exp)))
    pattern, in_maps, gather = _prep_inputs(
        np.asarray(spikes, dtype=np.float32),
        np.asarray(weights, dtype=np.float32),
        np.asarray(mask, dtype=np.float32),
        np.asarray(scale_exp),
    )
    nc = _get_program(thr, pattern)
    res = run_bass_kernel_spmd(
        nc, in_maps, core_ids=list(range(NCORES)), **run_kwargs
    )
    full = np.zeros((B, COUT, T), dtype=np.float32)
    for core in range(NCORES):
        bh, cq = divmod(core, NQ)
        blks = [
            np.asarray(res.results[core][f"out{j}"]) for j in range(NBLK)
        ]  # each [cout_lo, t, m, b]
        a = np.concatenate(blks, axis=1)  # [cout_lo, T, m, b]
        # -> [b, m, cout_lo, T] -> [b_loc, qloc, T]
        a = a.transpose(3, 2, 0, 1).reshape(BLOC, QLOC, T)
        full[bh * BLOC : (bh + 1) * BLOC, gather[core]] = a
    if run_kwargs:
        return full, res
    return full
